# revision 39
# baseline (speedup 1.0000x reference)
"""Trainium2 Bass kernel for nn_AFModule (adaptive-focus video module).

Data-parallel across 8 NeuronCores on the batch axis: 16 images = 2 videos
x 8 segments per core. All conv/BN/gate weights are BN-folded on the host
and replicated to every core. Convs run on TensorE as per-tap accumulated
matmuls (bf16 inputs, f32 PSUM); 64-channel convs pack two taps per matmul
using a row-shifted copy of the input on partitions 64..127.
"""
import numpy as np
from contextlib import ExitStack

import concourse.bass as bass
import concourse.bacc as bacc
import concourse.tile as tile
from concourse import mybir
from concourse.bass_utils import run_bass_kernel_spmd

FP = mybir.dt.float32
BF = mybir.dt.bfloat16
ACTF = mybir.ActivationFunctionType
ALU = mybir.AluOpType
AX = mybir.AxisListType

NCORES = 8
B, S, CIN, CAMP, COUT, H = 128, 8, 64, 64, 128, 56
HP = H + 2            # padded 58
HO = H // 2           # 28
HOP = HO + 2          # 30
IPC = B // NCORES     # 16 images per core
NPIX = H * H          # 3136
NPIXO = HO * HO       # 784
RC = 8                # output rows per full-res chunk
NCH = H // RC         # 7 chunks
NF = RC * H           # 448 free elems per chunk
RCO = 14              # output rows per low-res chunk
NCHO = HO // RCO      # 2 chunks
NFO = RCO * HO        # 392


def build_nc(n_img=IPC):
    nc = bacc.Bacc()
    x_d = nc.declare_dram_parameter("x", [IPC, CIN, H, H], FP, isOutput=False)
    gum_d = nc.declare_dram_parameter("gumbd", [1, IPC], FP, isOutput=False)
    wp = {}
    for name, shp in [
        ("w_a1p", [3, 2 * CIN, CAMP]), ("w_a1s", [3, CIN, CAMP]),
        ("w_a2p", [3, 2 * CAMP, CAMP]), ("w_a2s", [3, CAMP, CAMP]),
        ("w_f1p", [3, 2 * CIN, COUT]), ("w_f1s", [3, CIN, COUT]),
        ("w_f2", [9, COUT, COUT]), ("w_fu1", [9, COUT, COUT]),
        ("w_fu2", [9, COUT, COUT]),
        ("w_rb", [CIN, COUT]), ("w_bt", [CAMP, COUT]),
    ]:
        wp[name] = nc.declare_dram_parameter(name, shp, BF, isOutput=False)
    for name, shp in [
        ("w_nav", [CIN, 2]), ("w_gsd", [2, S]),
        ("w_at1", [COUT, COUT // 16]), ("w_at2", [COUT // 16, COUT]),
        ("b_a1", [CAMP, 1]), ("b_a2", [CAMP, 1]), ("b_bt", [COUT, 1]),
        ("b_rb", [COUT, 1]), ("b_f1", [COUT, 1]), ("b_f2", [COUT, 1]),
        ("b_fu1", [COUT, 1]), ("b_fu2", [COUT, 1]), ("b_nav", [2, 1]),
    ]:
        wp[name] = nc.declare_dram_parameter(name, shp, FP, isOutput=False)
    out_d = nc.declare_dram_parameter("out", [IPC, COUT, H, H], FP, isOutput=True)
    mask_d = nc.declare_dram_parameter("mask", [IPC], FP, isOutput=True)

    with tile.TileContext(nc) as tc, ExitStack() as ctx:
        wpool = ctx.enter_context(tc.tile_pool(name="wts", bufs=1))
        pers = ctx.enter_context(tc.tile_pool(name="pers", bufs=1))
        act = ctx.enter_context(tc.tile_pool(name="act", bufs=3))
        sml = ctx.enter_context(tc.tile_pool(name="sml", bufs=3))
        ps_c = ctx.enter_context(tc.tile_pool(name="psc", bufs=6, space="PSUM"))
        ps_t = ctx.enter_context(tc.tile_pool(name="pst", bufs=2, space="PSUM"))

        padp = ctx.enter_context(tc.tile_pool(name="padp", bufs=2))

        def zero_borders(t, parts, hp):
            # rows 0 and hp-1, plus the (r, hp-1)/(r+1, 0) column strips
            nc.gpsimd.memset(t[0:parts, 0:hp + 1], 0.0)
            nc.gpsimd.memset(t[0:parts, (hp - 1) * hp:hp * hp], 0.0)
            n = hp - 2
            strip = t[0:parts, 2 * hp - 1:2 * hp - 1 + n * hp] \
                .rearrange("p (r c) -> p r c", r=n)[:, :, 0:2]
            nc.gpsimd.memset(strip, 0.0)

        # preload image 0 before everything; image 1 after the weights its
        # front stage needs, so the first-image critical path is short
        preloaded = {}

        def preload_x(i):
            # row-split DMAs so the first ample chunk can start before the
            # whole image has landed
            xpad = padp.tile([2 * CIN, HP * HP], BF, tag="xpad")
            zero_borders(xpad, 2 * CIN, HP)
            nc.gpsimd.memset(xpad[CIN:2 * CIN, 56 * HP:57 * HP], 0.0)
            x3 = xpad[:].rearrange("p (h w) -> p h w", h=HP)
            hh = H // 2
            xr = x_d[i][:]
            nc.gpsimd.dma_start(x3[:CIN, 1:1 + hh, 1:1 + H], xr[:, 0:hh, :])
            nc.gpsimd.dma_start(x3[CIN:2 * CIN, 0:hh, 1:1 + H], xr[:, 0:hh, :])
            nc.gpsimd.dma_start(x3[:CIN, 1 + hh:1 + H, 1:1 + H], xr[:, hh:H, :])
            nc.gpsimd.dma_start(x3[CIN:2 * CIN, hh:H, 1:1 + H], xr[:, hh:H, :])
            preloaded[i] = xpad

        # weights arrive pre-cast to bf16, so they ride the HWDGE rings
        # (sync) and stay off the gpsimd queues that carry x
        wt = {}

        def load_w(name):
            shp = list(wp[name].shape)
            if len(shp) == 3:
                t = wpool.tile([shp[1], shp[0] * shp[2]], BF, tag=name)
                nc.sync.dma_start(
                    t[:].rearrange("k (t m) -> k t m", t=shp[0]),
                    wp[name][:].rearrange("t k m -> k t m"))
            else:
                t = wpool.tile(shp, BF, tag=name)
                nc.sync.dma_start(t[:], wp[name][:])
            wt[name] = t

        preload_x(0)
        for name in ["w_a1p", "w_a1s", "w_a2p", "w_a2s", "w_bt",
                     "w_f1p", "w_f1s", "w_f2", "w_rb"]:
            load_w(name)
        if n_img > 1:
            preload_x(1)
        for name in ["w_fu1", "w_fu2"]:
            load_w(name)
        # small f32 weights/biases
        for name in ["w_nav", "w_gsd", "w_at1", "w_at2", "b_a1", "b_a2", "b_bt",
                     "b_rb", "b_f1", "b_f2", "b_fu1", "b_fu2", "b_nav"]:
            t = wpool.tile(list(wp[name].shape), FP, tag=name)
            nc.sync.dma_start(t[:], wp[name][:])
            wt[name] = t
        gumbd = wpool.tile([1, IPC], FP, tag="gumbd")
        nc.sync.dma_start(gumbd[:], gum_d[:])
        ones = wpool.tile([1, 128], FP, tag="ones")
        nc.vector.memset(ones[:], 1.0)

        def wslice(name, j, m):
            return wt[name][:, j * m:(j + 1) * m]

        y64 = pers.tile([CIN, IPC], FP, tag="y64")
        mrow = pers.tile([1, IPC], FP, tag="mrow")
        nc.vector.memset(mrow[:], 0.0)

        def front_a(i):
            """input DMAs, ample chain (through conv2) -> state dict."""
            if i in preloaded:
                xpad = preloaded[i]
                x3 = xpad[:].rearrange("p (h w) -> p h w", h=HP)
            else:
                xpad = padp.tile([2 * CIN, HP * HP], BF, tag="xpad")
                zero_borders(xpad, 2 * CIN, HP)
                nc.gpsimd.memset(xpad[CIN:2 * CIN, 56 * HP:57 * HP], 0.0)
                x3 = xpad[:].rearrange("p (h w) -> p h w", h=HP)
                # cast-DMAs into padded interior; partitions 64..127 hold the
                # same data shifted up one row (pairs two conv taps per matmul)
                nc.gpsimd.dma_start(x3[:CIN, 1:1 + H, 1:1 + H], x_d[i])
                nc.gpsimd.dma_start(x3[CIN:2 * CIN, 0:H, 1:1 + H], x_d[i])
            hapad = padp.tile([2 * CAMP, HOP * HOP], BF, tag="hapad")
            hfpad = padp.tile([COUT, HP * HP], BF, tag="hfpad")
            zero_borders(hapad, CAMP, HOP)
            nc.gpsimd.memset(hapad[CAMP:2 * CAMP, 28 * HOP:HOP * HOP], 0.0)
            zero_borders(hfpad, COUT, HP)
            ha3 = hapad[:].rearrange("p (h w) -> p h w", h=HOP)
            hf3 = hfpad[:].rearrange("p (h w) -> p h w", h=HP)

            # ample conv1 (stride 2)
            for c in range(NCHO):
                r0 = c * RCO
                ps = ps_c.tile([CAMP, NFO], FP, tag="ps")
                for dx in range(3):
                    rhs = x3[:, 2 * r0:2 * r0 + 2 * RCO:2, dx:dx + H:2]
                    nc.tensor.matmul(ps[:], wslice("w_a1p", dx, CAMP), rhs,
                                     start=(dx == 0), stop=False)
                for dx in range(3):
                    rhs = x3[:CIN, 2 * r0 + 2:2 * r0 + 2 + 2 * RCO:2, dx:dx + H:2]
                    nc.tensor.matmul(ps[:], wslice("w_a1s", dx, CAMP), rhs,
                                     start=False, stop=(dx == 2))
                nc.scalar.activation(
                    ha3[:CAMP, 1 + r0:1 + r0 + RCO, 1:1 + HO],
                    ps[:].rearrange("p (h w) -> p h w", h=RCO),
                    ACTF.Relu, bias=wt["b_a1"][:, 0:1])
            nc.vector.tensor_copy(hapad[CAMP:2 * CAMP, 0:28 * HOP],
                                  hapad[0:CAMP, HOP:29 * HOP])
            return dict(i=i, xpad=xpad, hapad=hapad, hfpad=hfpad, x3=x3,
                        ha3=ha3, hf3=hf3)

        def front_a2(st):
            """avgpool + ample conv2 + xlit eviction."""
            i, x3, ha3 = st["i"], st["x3"], st["ha3"]
            # avgpool 3x3 s2 on VectorE: 8 chained tap adds (walrus requires
            # equal base partitions when both TT inputs are in SBUF)
            avg = act.tile([CAMP, NPIXO], FP, tag="avg")
            av3 = avg[:].rearrange("p (h w) -> p h w", h=HO)
            sdx = act.tile([CAMP, NPIXO], FP, tag="sdx")
            first = True
            for dy in range(3):
                for dx in range(3):
                    tap = x3[:CIN, dy:dy + 2 * HO:2, dx:dx + H:2]
                    if first:
                        prev_tap = tap
                        first = False
                        continue
                    if prev_tap is not None:
                        nc.vector.tensor_add(av3[:, :, :], prev_tap, tap)
                        prev_tap = None
                    else:
                        nc.vector.tensor_add(av3[:, :, :], av3[:, :, :], tap)
            nc.vector.tensor_scalar(avg[:], avg[:], 1.0 / 9.0, None, ALU.mult)

            # ample conv2 (accumulated in PSUM); avgpool residual merged in
            # on VectorE during the eviction
            xlit = act.tile([CAMP, NPIXO], BF, tag="xlit")
            for c in range(NCHO):
                r0 = c * RCO
                ps = ps_c.tile([CAMP, NFO], FP, tag="ps")
                for dx in range(3):
                    rhs = ha3[:, r0:r0 + RCO, dx:dx + HO]
                    nc.tensor.matmul(ps[:], wslice("w_a2p", dx, CAMP), rhs,
                                     start=(dx == 0), stop=False)
                for dx in range(3):
                    rhs = ha3[:CAMP, r0 + 2:r0 + 2 + RCO, dx:dx + HO]
                    nc.tensor.matmul(ps[:], wslice("w_a2s", dx, CAMP), rhs,
                                     start=False, stop=(dx == 2))
                sl = slice(c * NFO, (c + 1) * NFO)
                nc.vector.tensor_add(sdx[:, sl], ps[:], avg[:, sl])
                nc.vector.tensor_scalar(xlit[:, sl], sdx[:, sl],
                                        wt["b_a2"][:, 0:1], None, ALU.add)
            st["xlit"] = xlit

        def front_b(st):
            """routing mask, base_transform, focal convs."""
            i, xpad, xlit, x3, hf3 = (st["i"], st["xpad"], st["xlit"],
                                      st["x3"], st["hf3"])
            # routing mask
            nc.vector.tensor_reduce(y64[:, i:i + 1], xpad[0:CIN, :], AX.X, ALU.add)
            psg = ps_t.tile([2, 1], FP, tag="pst")
            nc.tensor.matmul(psg[:], wt["w_nav"][:], y64[:, i:i + 1],
                             start=True, stop=True)
            gvec = sml.tile([2, 1], FP, tag="gvec")
            nc.scalar.activation(gvec[:], psg[:], ACTF.Relu, bias=wt["b_nav"][:, 0:1])
            psl = ps_t.tile([1, 1], FP, tag="pst")
            s = i % S
            nc.tensor.matmul(psl[:], wt["w_gsd"][:, s:s + 1], gvec[:],
                             start=True, stop=True)
            dsc = sml.tile([1, 1], FP, tag="dsc")
            nc.vector.tensor_add(dsc[:], psl[:], gumbd[0:1, i:i + 1])
            nc.vector.tensor_scalar(mrow[0:1, i:i + 1], dsc[:], 0.0, None, ALU.is_gt)
            psb = ps_t.tile([128, 1], FP, tag="pst")
            nc.tensor.matmul(psb[:], ones[:], mrow[0:1, i:i + 1], start=True, stop=True)
            mcol = sml.tile([128, 1], FP, tag="mcol")
            nc.scalar.activation(mcol[:], psb[:], ACTF.Copy)
            bias_hf = sml.tile([COUT, 1], FP, tag="bias_hf")
            nc.vector.tensor_scalar(bias_hf[:], wt["b_f1"][:], mcol[0:COUT, 0:1],
                                    None, ALU.mult)
            bias_xb = sml.tile([COUT, 1], FP, tag="bias_xb")
            nc.vector.tensor_scalar(bias_xb[:], wt["b_f2"][:], mcol[0:COUT, 0:1],
                                    wt["b_rb"][:, 0:1], ALU.mult, ALU.add)

            # base_transform 1x1 (64 -> 128)
            xl = act.tile([COUT, NPIXO], FP, tag="xl")
            sxl = sml.tile([COUT, NCHO], FP, tag="sxl")
            for c in range(NCHO):
                ps = ps_c.tile([COUT, NFO], FP, tag="ps")
                nc.tensor.matmul(ps[:], wt["w_bt"][:], xlit[:, c * NFO:(c + 1) * NFO],
                                 start=True, stop=True)
                nc.scalar.activation(xl[:, c * NFO:(c + 1) * NFO], ps[:], ACTF.Identity,
                                     bias=wt["b_bt"][:, 0:1],
                                     accum_out=sxl[:, c:c + 1])

            # focal conv1 (mask gating folded into the eviction scale)
            for c in range(NCH):
                r0 = c * RC
                ps = ps_c.tile([COUT, NF], FP, tag="ps")
                for dx in range(3):
                    rhs = x3[:, r0:r0 + RC, dx:dx + H]
                    nc.tensor.matmul(ps[:], wslice("w_f1p", dx, COUT), rhs,
                                     start=(dx == 0), stop=False)
                for dx in range(3):
                    rhs = x3[:CIN, r0 + 2:r0 + 2 + RC, dx:dx + H]
                    nc.tensor.matmul(ps[:], wslice("w_f1s", dx, COUT), rhs,
                                     start=False, stop=(dx == 2))
                nc.scalar.activation(
                    hf3[:, 1 + r0:1 + r0 + RC, 1:1 + H],
                    ps[:].rearrange("p (h w) -> p h w", h=RC),
                    ACTF.Relu, bias=bias_hf[:, 0:1], scale=mcol[0:COUT, 0:1])

            # focal conv2 + res_b (1x1) accumulated; evict -> x_big
            xb = act.tile([COUT, NPIX], FP, tag="xb")
            sxb = sml.tile([COUT, NCH], FP, tag="sxb")
            for c in range(NCH):
                r0 = c * RC
                ps = ps_c.tile([COUT, NF], FP, tag="ps")
                for t in range(9):
                    dy, dx = t // 3, t % 3
                    rhs = hf3[:, r0 + dy:r0 + dy + RC, dx:dx + H]
                    nc.tensor.matmul(ps[:], wslice("w_f2", t, COUT), rhs,
                                     start=(t == 0), stop=False)
                nc.tensor.matmul(ps[:], wt["w_rb"][:],
                                 x3[:CIN, 1 + r0:1 + r0 + RC, 1:1 + H],
                                 start=False, stop=True)
                nc.scalar.activation(xb[:, c * NF:(c + 1) * NF], ps[:], ACTF.Identity,
                                     bias=bias_xb[:, 0:1],
                                     accum_out=sxb[:, c:c + 1])
            st.update(xb=xb, xl=xl, sxl=sxl, sxb=sxb)

        def back_se_dve(st):
            """SE stats reductions (VectorE) - emitted early."""
            sxl, sxb = st["sxl"], st["sxb"]
            t1 = sml.tile([COUT, 1], FP, tag="t1")
            nc.vector.tensor_reduce(t1[:], sxl[:], AX.X, ALU.add)
            t2 = sml.tile([COUT, 1], FP, tag="t2")
            nc.vector.tensor_reduce(t2[:], sxb[:], AX.X, ALU.add)
            yse = sml.tile([COUT, 1], FP, tag="yse")
            nc.vector.tensor_scalar(yse[:], t1[:], 4.0, None, ALU.mult)
            nc.vector.tensor_add(yse[:], yse[:], t2[:])
            st["yse"] = yse

        def back_se(st):
            """SE attention matmuls (tiny, TensorE + ScalarE)."""
            yse = st["yse"]
            psa = ps_t.tile([COUT // 16, 1], FP, tag="pst")
            nc.tensor.matmul(psa[:], wt["w_at1"][:], yse[:], start=True, stop=True)
            zat = sml.tile([COUT // 16, 1], FP, tag="zat")
            nc.scalar.activation(zat[:], psa[:], ACTF.Relu)
            psb2 = ps_t.tile([COUT, 1], FP, tag="pst")
            nc.tensor.matmul(psb2[:], wt["w_at2"][:], zat[:], start=True, stop=True)
            att = sml.tile([COUT, 1], FP, tag="att")
            nc.scalar.activation(att[:], psb2[:], ACTF.Sigmoid)
            st["att"] = att

        def back_rest(st):
            """fusion, fu residual block, output DMA."""
            i, xb, xl, att = st["i"], st["xb"], st["xl"], st["att"]
            opad = padp.tile([COUT, HP * HP], BF, tag="outpad")
            hupad = padp.tile([COUT, HP * HP], BF, tag="hupad")
            zero_borders(opad, COUT, HP)
            zero_borders(hupad, COUT, HP)
            o3 = opad[:].rearrange("p (h w) -> p h w", h=HP)
            hu3 = hupad[:].rearrange("p (h w) -> p h w", h=HP)

            # fusion: out = relu(x_big + att*(xl_up - x_big)) -> out_pad bf16
            xl3 = xl[:].rearrange("p (h w) -> p h w", h=HO)
            for c in range(NCH):
                r0 = c * RC
                dt_ = act.tile([COUT, NF], FP, tag="dt")
                src = xl3[:, r0 // 2:r0 // 2 + RC // 2, :, None] \
                    .broadcast_to([COUT, RC // 2, HO, 2])
                d5 = dt_[:].rearrange("p (h a w b) -> p h a w b",
                                      h=RC // 2, a=2, b=2)
                x5 = xb[:, c * NF:(c + 1) * NF].rearrange(
                    "p (h a w b) -> p h a w b", h=RC // 2, a=2, b=2)
                for a in range(2):
                    nc.vector.tensor_sub(d5[:, :, a], src, x5[:, :, a])
                nc.vector.tensor_scalar(dt_[:], dt_[:], att[:, 0:1], None, ALU.mult)
                nc.vector.tensor_add(dt_[:], dt_[:], xb[:, c * NF:(c + 1) * NF])
                nc.scalar.activation(
                    o3[:, 1 + r0:1 + r0 + RC, 1:1 + H],
                    dt_[:].rearrange("p (h w) -> p h w", h=RC), ACTF.Relu)

            # fusion block conv1
            for c in range(NCH):
                r0 = c * RC
                ps = ps_c.tile([COUT, NF], FP, tag="ps")
                for t in range(9):
                    dy, dx = t // 3, t % 3
                    rhs = o3[:, r0 + dy:r0 + dy + RC, dx:dx + H]
                    nc.tensor.matmul(ps[:], wslice("w_fu1", t, COUT), rhs,
                                     start=(t == 0), stop=(t == 8))
                nc.scalar.activation(
                    hu3[:, 1 + r0:1 + r0 + RC, 1:1 + H],
                    ps[:].rearrange("p (h w) -> p h w", h=RC),
                    ACTF.Relu, bias=wt["b_fu1"][:, 0:1])

            # fusion block conv2 + residual + relu -> DRAM
            for c in range(NCH):
                r0 = c * RC
                ps = ps_c.tile([COUT, NF], FP, tag="ps")
                for t in range(9):
                    dy, dx = t // 3, t % 3
                    rhs = hu3[:, r0 + dy:r0 + dy + RC, dx:dx + H]
                    nc.tensor.matmul(ps[:], wslice("w_fu2", t, COUT), rhs,
                                     start=(t == 0), stop=(t == 8))
                ft = act.tile([COUT, NF], FP, tag="ft")
                nc.vector.tensor_add(ft[:], ps[:],
                                     o3[:, 1 + r0:1 + r0 + RC, 1:1 + H])
                fin = act.tile([COUT, NF], FP, tag="fin")
                nc.scalar.activation(fin[:], ft[:], ACTF.Relu,
                                     bias=wt["b_fu2"][:, 0:1])
                nc.sync.dma_start(out_d[i][:, r0:r0 + RC, :],
                                  fin[:].rearrange("p (h w) -> p h w", h=RC))

        # skew-1 software pipeline: image i's ample stage, then image i-1's
        # tiny SE chain (while ScalarE drains evictions), then image i's
        # focal stage, then image i-1's fusion + fu block
        prev = None
        for i in range(n_img):
            if prev is not None:
                back_se_dve(prev)
            st = front_a(i)
            front_a2(st)
            if prev is not None:
                back_se(prev)
            front_b(st)
            if prev is not None:
                back_rest(prev)
            prev = st
        if prev is not None:
            back_se_dve(prev)
            back_se(prev)
            back_rest(prev)

        nc.sync.dma_start(mask_d[:], mrow[0:1, :])
    nc.finalize()
    return nc


def _prep_host(inputs):
    f = {k: np.asarray(v, np.float32) for k, v in inputs.items()}

    def tap_lhsT(w):  # w: [O, I, 3, 3] (BN-folded) -> pairs + singles
        pairs = np.stack([np.concatenate([w[:, :, 0, dx].T, w[:, :, 1, dx].T], 0)
                          for dx in range(3)])
        singles = np.stack([w[:, :, 2, dx].T for dx in range(3)])
        return np.ascontiguousarray(pairs), np.ascontiguousarray(singles)

    wa1 = f["ba_w1"] * f["ba_g1"][:, None, None, None]
    wa2 = f["ba_w2"] * f["ba_g2"][:, None, None, None]
    wf1 = f["bf_w1"] * f["bf_g1"][:, None, None, None]
    wf2 = f["bf_w2"] * f["bf_g2"][:, None, None, None]
    wfu1 = f["fu_w1"] * f["fu_g1"][:, None, None, None]
    wfu2 = f["fu_w2"] * f["fu_g2"][:, None, None, None]
    a1p, a1s = tap_lhsT(wa1)
    a2p, a2s = tap_lhsT(wa2)
    f1p, f1s = tap_lhsT(wf1)

    def full_taps(w):  # [9, I, O]
        return np.ascontiguousarray(
            np.stack([w[:, :, t // 3, t % 3].T for t in range(9)]))

    com = dict(
        w_a1p=a1p, w_a1s=a1s, w_a2p=a2p, w_a2s=a2s,
        w_f1p=f1p, w_f1s=f1s, w_f2=full_taps(wf2),
        w_fu1=full_taps(wfu1), w_fu2=full_taps(wfu2),
        w_rb=np.ascontiguousarray((f["bf_wd"] * f["bf_gd"][:, None]).T),
        w_bt=np.ascontiguousarray((f["bt_w"] * f["bt_g"][:, None]).T),
        w_nav=np.ascontiguousarray(
            (f["nav_w"] * f["nav_bn_g"][:, None] / NPIX).T),
        w_gsd=np.ascontiguousarray(
            np.stack([f["gs_w"][s, 1, :] - f["gs_w"][s, 0, :]
                      for s in range(S)], 1)),
        w_at1=np.ascontiguousarray(f["att_w1"].T / NPIX),
        w_at2=np.ascontiguousarray(f["att_w2"].T),
        b_a1=f["ba_b1"][:, None], b_a2=f["ba_b2"][:, None],
        b_bt=f["bt_b"][:, None], b_rb=f["bf_bd"][:, None],
        b_f1=f["bf_b1"][:, None], b_f2=f["bf_b2"][:, None],
        b_fu1=f["fu_b1"][:, None], b_fu2=f["fu_b2"][:, None],
        b_nav=f["nav_bn_b"][:, None],
    )
    import ml_dtypes
    bf_names = {"w_a1p", "w_a1s", "w_a2p", "w_a2s", "w_f1p", "w_f1s",
                "w_f2", "w_fu1", "w_fu2", "w_rb", "w_bt"}
    com = {k: np.ascontiguousarray(
        v, dtype=ml_dtypes.bfloat16 if k in bf_names else np.float32)
        for k, v in com.items()}

    gb = f["gs_b"][None, :, :] + f["gumbel"]          # [16, 8, 2]
    gbd = gb[:, :, 1] - gb[:, :, 0]                   # [16, 8]
    nv = IPC // S
    in_maps = []
    for m in range(NCORES):
        d = dict(com)
        d["x"] = np.ascontiguousarray(f["x"][m * IPC:(m + 1) * IPC])
        d["gumbd"] = np.ascontiguousarray(
            gbd[m * nv:(m + 1) * nv].reshape(1, IPC))
        in_maps.append(d)
    return in_maps


def kernel(**inputs):
    in_maps = _prep_host(inputs)
    nc = build_nc()
    res = run_bass_kernel_spmd(nc, in_maps, list(range(NCORES))).results
    out = np.concatenate([r["out"] for r in res], 0)
    mask = np.concatenate([r["mask"] for r in res], 0).reshape(B, 1, 1, 1)
    return out.astype(np.float32), mask.astype(np.float32)
